# revision 23
# baseline (speedup 1.0000x reference)
import os
import sys

for _p in ("/opt/trn_rl_repo", "/root/.axon_site/_ro/trn_rl_repo"):
    if os.path.isdir(_p) and _p not in sys.path:
        sys.path.insert(0, _p)

from contextlib import ExitStack

import numpy as np

import concourse.bass as bass
import concourse.tile as tile
from concourse import bacc, mybir
from concourse.bass_utils import run_bass_kernel_spmd
from concourse.masks import make_identity

# Problem shapes (hardcoded per spec): cross-attention
#   q = input1 @ W^T + b ; attn = softmax(q @ input2^T) ;
#   o1 = attn @ input2 ; o2 = attn^T @ input1
B, N1, N2, D = 8, 2048, 2048, 512

PT = 128            # partition tile
NT = N1 // PT       # 16 query row-tiles
MT = N2 // PT       # 16 key row-tiles
KT = D // PT        # 4 contraction tiles over D
CHUNK = 512         # moving-dim chunk (PSUM bank = 512 fp32)
MC = N2 // CHUNK    # 4 chunks of keys

F32 = mybir.dt.float32
F32R = mybir.dt.float32r
MM_DT = mybir.dt.float32r  # S-path matmul dtype: float32 (exact) or float32r (fast)
BF16 = mybir.dt.bfloat16
AF = mybir.ActivationFunctionType
AX = mybir.AxisListType
ALU = mybir.AluOpType


def _build():
    """One NeuronCore program: full cross-attention for ONE batch sample.

    Math (A = input1[b] [N1,D], Bm = input2[b] [N2,D], W [D,D], bvec [D]):
      C^T[d,m]   = sum_o W[o,d] * Bm[m,o]          (projected keys, W natural as lhsT)
      bias_row[m]= sum_o bvec[o] * Bm[m,o]
      S[n,m]     = sum_d A[n,d] * C[m,d] + bias_row[m]   (bias via K=1 matmul)
      P[n,m]     = exp(S - rowmax(S))  (bf16), rowsum via ACT accum_out
      o1[n,d]    = (1/rowsum[n]) * sum_m P^T[m,n] * Bm[m,d]   (P^T via PE transpose)
      o2[m,d]    = sum_n P[n,m] * (A[n,d]/rowsum[n])
    """
    nc = bacc.Bacc("TRN2", target_bir_lowering=False, debug=False, num_devices=B)
    a_d = nc.dram_tensor("a", [N1, D], MM_DT, kind="ExternalInput").ap()
    b_d = nc.dram_tensor("bm", [N2, D], MM_DT, kind="ExternalInput").ap()
    w_d = nc.dram_tensor("w", [D, D], MM_DT, kind="ExternalInput").ap()
    bv_d = nc.dram_tensor("bvec", [D], MM_DT, kind="ExternalInput").ap()
    ones_d = nc.dram_tensor("ones", [1, PT], MM_DT, kind="ExternalInput").ap()
    eye_d = nc.dram_tensor("eye", [PT, PT], MM_DT, kind="ExternalInput").ap()
    o1_d = nc.dram_tensor("o1", [N1, D], F32, kind="ExternalOutput").ap()
    o2_d = nc.dram_tensor("o2", [N2, D], F32, kind="ExternalOutput").ap()

    with tile.TileContext(nc) as tc, ExitStack() as big:
        const = big.enter_context(tc.tile_pool(name="const", bufs=1))
        ident = const.tile([PT, PT], MM_DT, name="ident", tag="ident")
        nc.sync.dma_start(ident[:], eye_d[:])
        ident_b = const.tile([PT, PT], BF16, name="identb", tag="identb")
        make_identity(nc, ident_b[:])
        ones_row = const.tile([1, PT], MM_DT, name="ones", tag="ones")
        nc.sync.dma_start(ones_row[:], ones_d[:])
        b_col = const.tile([PT, KT], MM_DT, name="bcol", tag="bcol")
        nc.sync.dma_start(b_col[:], bv_d.rearrange("(k p) -> p k", p=PT))
        bias_row = const.tile([1, N2], MM_DT, name="biasrow", tag="biasrow")

        stats = big.enter_context(tc.tile_pool(name="stats", bufs=1))
        recip_all = stats.tile([PT, NT], F32, name="recip", tag="recip")

        at_pool = big.enter_context(tc.tile_pool(name="atp", bufs=1))
        AT = [at_pool.tile([PT, N1], MM_DT, name=f"at{k}", tag=f"at{k}") for k in range(KT)]
        ct_pool = big.enter_context(tc.tile_pool(name="ctp", bufs=1))
        CT = [ct_pool.tile([PT, N2], MM_DT, name=f"ct{k}", tag=f"ct{k}") for k in range(KT)]
        bbf_pool = big.enter_context(tc.tile_pool(name="bbfp", bufs=1))
        Bbf = [bbf_pool.tile([PT, D], BF16, name=f"bbf{t}", tag=f"bbf{t}") for t in range(MT)]

        # ---------------- phase 0: load + transposes + projection ----------
        with ExitStack() as ph0:
            wp = ph0.enter_context(tc.tile_pool(name="wp", bufs=1))
            Wt = [wp.tile([PT, D], MM_DT, name=f"w{k}", tag=f"w{k}") for k in range(KT)]
            for k in range(KT):
                nc.sync.dma_start(Wt[k][:], w_d[k * PT:(k + 1) * PT, :])
            btp = ph0.enter_context(tc.tile_pool(name="btp", bufs=1))
            BT = [btp.tile([PT, N2], MM_DT, name=f"bt{k}", tag=f"bt{k}") for k in range(KT)]
            ldp = ph0.enter_context(tc.tile_pool(name="ldp", bufs=1))
            ps0 = ph0.enter_context(tc.tile_pool(name="ps0", bufs=1, space="PSUM"))

            for t in range(MT):
                btile = ldp.tile([PT, D], MM_DT, name="ld", tag="ld", bufs=4)
                nc.sync.dma_start(btile[:], b_d[t * PT:(t + 1) * PT, :])
                nc.vector.tensor_copy(Bbf[t][:], btile[:])
                for k in range(KT):
                    trp = ps0.tile([PT, PT], MM_DT, name="tr", tag="tr", bufs=2)
                    nc.tensor.transpose(trp[:], btile[:, k * PT:(k + 1) * PT], ident[:])
                    nc.vector.tensor_copy(BT[k][:, t * PT:(t + 1) * PT], trp[:])
            for t in range(NT):
                atile = ldp.tile([PT, D], MM_DT, name="ld", tag="ld", bufs=4)
                nc.sync.dma_start(atile[:], a_d[t * PT:(t + 1) * PT, :])
                for k in range(KT):
                    trp = ps0.tile([PT, PT], MM_DT, name="tr", tag="tr", bufs=2)
                    nc.tensor.transpose(trp[:], atile[:, k * PT:(k + 1) * PT], ident[:])
                    nc.vector.tensor_copy(AT[k][:, t * PT:(t + 1) * PT], trp[:])

            # C^T[d2, m] = sum_o W[o, d2] * BT[o, m]
            for k2 in range(KT):
                for mc in range(MC):
                    cps = ps0.tile([PT, CHUNK], F32, name="mm", tag="mm", bufs=5)
                    for ko in range(KT):
                        nc.tensor.matmul(
                            cps[:],
                            Wt[ko][:, k2 * PT:(k2 + 1) * PT],
                            BT[ko][:, mc * CHUNK:(mc + 1) * CHUNK],
                            start=(ko == 0), stop=(ko == KT - 1),
                        )
                    nc.scalar.copy(CT[k2][:, mc * CHUNK:(mc + 1) * CHUNK], cps[:])
            # bias_row[m] = sum_o bvec[o] * BT[o, m]
            for mc in range(MC):
                bps = ps0.tile([1, CHUNK], F32, name="bias", tag="bias", bufs=1)
                for ko in range(KT):
                    nc.tensor.matmul(
                        bps[:],
                        b_col[:, ko:ko + 1],
                        BT[ko][:, mc * CHUNK:(mc + 1) * CHUNK],
                        start=(ko == 0), stop=(ko == KT - 1),
                    )
                nc.vector.tensor_copy(bias_row[0:1, mc * CHUNK:(mc + 1) * CHUNK], bps[:])

        # ---------------- phase 1: S' -> P (bf16), row stats, A_scaled ------
        p_pool = big.enter_context(tc.tile_pool(name="pp", bufs=1))
        Pt = [p_pool.tile([PT, N2], BF16, name=f"p{t}", tag=f"p{t}") for t in range(NT)]
        asc_pool = big.enter_context(tc.tile_pool(name="ascp", bufs=1))
        Asc = [asc_pool.tile([PT, D], BF16, name=f"asc{t}", tag=f"asc{t}") for t in range(NT)]
        with ExitStack() as ph1:
            ps1 = ph1.enter_context(tc.tile_pool(name="ps1", bufs=1, space="PSUM"))
            smp = ph1.enter_context(tc.tile_pool(name="smp", bufs=1))
            ld1 = ph1.enter_context(tc.tile_pool(name="ld1", bufs=1))
            for nt in range(NT):
                spsums = []
                for mc in range(MC):
                    sps = ps1.tile([PT, CHUNK], F32, name="s", tag="s", bufs=8)
                    for k in range(KT):
                        nc.tensor.matmul(
                            sps[:],
                            AT[k][:, nt * PT:(nt + 1) * PT],
                            CT[k][:, mc * CHUNK:(mc + 1) * CHUNK],
                            start=(k == 0), stop=False,
                        )
                    nc.tensor.matmul(
                        sps[:],
                        ones_row[:],
                        bias_row[0:1, mc * CHUNK:(mc + 1) * CHUNK],
                        start=False, stop=True,
                    )
                    spsums.append(sps)
                rms = []
                for mc in range(MC):
                    rm = smp.tile([PT, 1], F32, name="rm", tag="rm", bufs=8)
                    nc.vector.tensor_reduce(rm[:], spsums[mc][:], axis=AX.X, op=ALU.max)
                    rms.append(rm)
                c01 = smp.tile([PT, 1], F32, name="c01", tag="c01", bufs=2)
                nc.vector.tensor_tensor(c01[:], rms[0][:], rms[1][:], op=ALU.max)
                c23 = smp.tile([PT, 1], F32, name="c23", tag="c23", bufs=2)
                nc.vector.tensor_tensor(c23[:], rms[2][:], rms[3][:], op=ALU.max)
                mx = smp.tile([PT, 1], F32, name="mx", tag="mx", bufs=2)
                nc.vector.tensor_tensor(mx[:], c01[:], c23[:], op=ALU.max)
                nmax = smp.tile([PT, 1], F32, name="nmax", tag="nmax", bufs=2)
                nc.vector.tensor_scalar_mul(nmax[:], mx[:], -1.0)
                sums = []
                for mc in range(MC):
                    sm = smp.tile([PT, 1], F32, name="sum", tag="sum", bufs=8)
                    nc.scalar.activation(
                        Pt[nt][:, mc * CHUNK:(mc + 1) * CHUNK],
                        spsums[mc][:], AF.Exp,
                        bias=nmax[:], scale=1.0, accum_out=sm[:],
                    )
                    sums.append(sm)
                s01 = smp.tile([PT, 1], F32, name="s01", tag="s01", bufs=2)
                nc.vector.tensor_add(s01[:], sums[0][:], sums[1][:])
                s23 = smp.tile([PT, 1], F32, name="s23", tag="s23", bufs=2)
                nc.vector.tensor_add(s23[:], sums[2][:], sums[3][:])
                stot = smp.tile([PT, 1], F32, name="stot", tag="stot", bufs=2)
                nc.vector.tensor_add(stot[:], s01[:], s23[:])
                nc.vector.reciprocal(recip_all[:, nt:nt + 1], stot[:])
                atile = ld1.tile([PT, D], MM_DT, name="ld", tag="ld", bufs=3)
                nc.sync.dma_start(atile[:], a_d[nt * PT:(nt + 1) * PT, :])
                nc.vector.tensor_scalar_mul(Asc[nt][:], atile[:], recip_all[:, nt:nt + 1])

        # ---------------- pass 2: o1 (needs P^T tiles) and o2 ---------------
        with ExitStack() as ph2:
            ps2 = ph2.enter_context(tc.tile_pool(name="ps2", bufs=1, space="PSUM"))
            ptp = ph2.enter_context(tc.tile_pool(name="ptp", bufs=1))
            outp = ph2.enter_context(tc.tile_pool(name="outp", bufs=1))
            for nt in range(NT):
                o1ps = ps2.tile([PT, D], F32, name="o1", tag="o1", bufs=2)
                for mt in range(MT):
                    trp = ps2.tile([PT, PT], BF16, name="tr", tag="tr", bufs=3)
                    nc.tensor.transpose(trp[:], Pt[nt][:, mt * PT:(mt + 1) * PT], ident_b[:])
                    pts = ptp.tile([PT, PT], BF16, name="pt", tag="pt", bufs=4)
                    nc.vector.tensor_copy(pts[:], trp[:])
                    nc.tensor.matmul(o1ps[:], pts[:], Bbf[mt][:],
                                     start=(mt == 0), stop=(mt == MT - 1))
                o1sb = outp.tile([PT, D], F32, name="o1sb", tag="o1sb", bufs=4)
                nc.scalar.mul(o1sb[:], o1ps[:], recip_all[:, nt:nt + 1])
                nc.sync.dma_start(o1_d[nt * PT:(nt + 1) * PT, :], o1sb[:])
            for mt in range(MT):
                o2ps = ps2.tile([PT, D], F32, name="o2", tag="o2", bufs=2)
                for nt in range(NT):
                    nc.tensor.matmul(o2ps[:], Pt[nt][:, mt * PT:(mt + 1) * PT], Asc[nt][:],
                                     start=(nt == 0), stop=(nt == NT - 1))
                o2sb = outp.tile([PT, D], F32, name="o2sb", tag="o2sb", bufs=4)
                nc.scalar.copy(o2sb[:], o2ps[:])
                nc.sync.dma_start(o2_d[mt * PT:(mt + 1) * PT, :], o2sb[:])
    nc.compile()
    return nc


_state = {}


def _get_nc():
    if "nc" not in _state:
        _state["nc"] = _build()
    return _state["nc"]


def _in_maps(input1, input2, W_w, W_b):
    return [
        {
            "a": np.ascontiguousarray(input1[bb], dtype=np.float32),
            "bm": np.ascontiguousarray(input2[bb], dtype=np.float32),
            "w": np.ascontiguousarray(W_w, dtype=np.float32),
            "bvec": np.ascontiguousarray(W_b, dtype=np.float32),
            "ones": np.ones((1, PT), dtype=np.float32),
            "eye": np.eye(PT, dtype=np.float32),
        }
        for bb in range(B)
    ]


def kernel(input1, input2, W_w, W_b):
    res = run_bass_kernel_spmd(
        _get_nc(), _in_maps(input1, input2, W_w, W_b), core_ids=list(range(B))
    )
    o1 = np.stack([r["o1"] for r in res.results])
    o2 = np.stack([r["o2"] for r in res.results])
    return o1, o2


def _pjrt_fn(nc, in_maps, donate=False):
    """Build a single-call jitted runner for `nc` (copy of run_bass_via_pjrt
    multi-core path, without donation so device inputs can be reused)."""
    import jax
    import numpy as np_
    from jax.sharding import Mesh, NamedSharding, PartitionSpec
    from jax.experimental.shard_map import shard_map

    from concourse import mybir as _mybir
    from concourse.bass2jax import (
        _bass_exec_p,
        install_neuronx_cc_hook,
        partition_id_tensor,
    )

    install_neuronx_cc_hook()
    partition_name = nc.partition_id_tensor.name if nc.partition_id_tensor else None

    in_names, out_names, out_avals, zero_outs = [], [], [], []
    for alloc in nc.m.functions[0].allocations:
        if not isinstance(alloc, _mybir.MemoryLocationSet):
            continue
        name = alloc.memorylocations[0].name
        if alloc.kind == "ExternalInput":
            if name != partition_name:
                in_names.append(name)
        elif alloc.kind == "ExternalOutput":
            out_names.append(name)
            shape = tuple(alloc.tensor_shape)
            dtype = _mybir.dt.np(alloc.dtype)
            out_avals.append(jax.core.ShapedArray(shape, dtype))
            zero_outs.append(np_.zeros(shape, dtype))

    all_in = list(in_names) + list(out_names)
    if partition_name is not None:
        all_in.append(partition_name)

    def _body(*args):
        operands = list(args)
        if partition_name is not None:
            operands.append(partition_id_tensor())
        outs = _bass_exec_p.bind(
            *operands,
            out_avals=tuple(out_avals),
            in_names=tuple(all_in),
            out_names=tuple(out_names),
            lowering_input_output_aliases=(),
            sim_require_finite=True,
            sim_require_nnan=True,
            nc=nc,
        )
        return tuple(outs)

    devices = jax.devices()[:B]
    mesh = Mesh(np_.asarray(devices), ("core",))
    nargs = len(in_names) + len(out_names)
    sh = NamedSharding(mesh, PartitionSpec("core"))
    fn = jax.jit(
        shard_map(
            _body, mesh=mesh,
            in_specs=(PartitionSpec("core"),) * nargs,
            out_specs=(PartitionSpec("core"),) * len(out_names),
            check_rep=False,
        ),
        **({"donate_argnums": tuple(range(len(in_names), nargs))} if donate else {}),
    )
    args = [
        jax.device_put(np_.concatenate([m[n] for m in in_maps], axis=0), sh)
        for n in in_names
    ] + [
        jax.device_put(np_.concatenate([z] * B, axis=0), sh) for z in zero_outs
    ]
    return fn, args, out_names, out_avals


def bench_hw(input1, input2, W_w, W_b, iters=30, reps=3):
    """Per-execution wall time of the jitted single NEFF call with
    device-resident inputs (includes PJRT dispatch overhead).
    Returns (best_ns, all_ns)."""
    import time

    import jax

    nc = _get_nc()
    fn, args, _, _ = _pjrt_fn(nc, _in_maps(input1, input2, W_w, W_b))
    r = fn(*args)
    jax.block_until_ready(r)
    times = []
    for _ in range(reps):
        t0 = time.perf_counter()
        for _ in range(iters):
            r = fn(*args)
        jax.block_until_ready(r)
        times.append((time.perf_counter() - t0) / iters * 1e9)
    return min(times), times


def bench_floor(iters=30, reps=3):
    """Dispatch floor: identical path with a trivial copy kernel."""
    import time

    import jax

    if "nc_tiny" not in _state:
        nc = bacc.Bacc("TRN2", target_bir_lowering=False, debug=False, num_devices=B)
        x_d = nc.dram_tensor("x", [PT, PT], F32, kind="ExternalInput").ap()
        y_d = nc.dram_tensor("y", [PT, PT], F32, kind="ExternalOutput").ap()
        with tile.TileContext(nc) as tc:
            with tc.tile_pool(name="tp", bufs=1) as tp:
                t = tp.tile([PT, PT], F32, name="t", tag="t")
                nc.sync.dma_start(t[:], x_d[:])
                nc.sync.dma_start(y_d[:], t[:])
        nc.compile()
        _state["nc_tiny"] = nc
    nc = _state["nc_tiny"]
    in_maps = [{"x": np.zeros((PT, PT), np.float32)} for _ in range(B)]
    fn, args, _, _ = _pjrt_fn(nc, in_maps)
    r = fn(*args)
    jax.block_until_ready(r)
    times = []
    for _ in range(reps):
        import time as _t
        t0 = _t.perf_counter()
        for _ in range(iters):
            r = fn(*args)
        jax.block_until_ready(r)
        times.append((_t.perf_counter() - t0) / iters * 1e9)
    return min(times), times


# revision 27
# speedup vs baseline: 3.6664x; 3.6664x over previous
import os
import sys

for _p in ("/opt/trn_rl_repo", "/root/.axon_site/_ro/trn_rl_repo"):
    if os.path.isdir(_p) and _p not in sys.path:
        sys.path.insert(0, _p)

from contextlib import ExitStack

import numpy as np

import concourse.bass as bass
import concourse.tile as tile
from concourse import bacc, mybir
from concourse.bass_utils import run_bass_kernel_spmd
from concourse.masks import make_identity

# Problem shapes (hardcoded per spec): cross-attention
#   q = input1 @ W^T + b ; attn = softmax(q @ input2^T) ;
#   o1 = attn @ input2 ; o2 = attn^T @ input1
B, N1, N2, D = 8, 2048, 2048, 512

PT = 128            # partition tile
NT = N1 // PT       # 16 query row-tiles
MT = N2 // PT       # 16 key row-tiles
KT = D // PT        # 4 contraction tiles over D
CHUNK = 512         # moving-dim chunk (PSUM bank = 512 fp32)
MC = N2 // CHUNK    # 4 chunks of keys

F32 = mybir.dt.float32
F32R = mybir.dt.float32r
MM_DT = mybir.dt.float32r  # S-path matmul dtype: float32 (exact) or float32r (fast)
BF16 = mybir.dt.float16  # storage dtype for P / B / A_scaled (fp16: same PE rate as bf16, 8x mantissa)
AF = mybir.ActivationFunctionType
AX = mybir.AxisListType
ALU = mybir.AluOpType


def _build(nreps=1):
    """One NeuronCore program: full cross-attention for ONE batch sample.

    Math (A = input1[b] [N1,D], Bm = input2[b] [N2,D], W [D,D], bvec [D]):
      C^T[d,m]   = sum_o W[o,d] * Bm[m,o]          (projected keys, W natural as lhsT)
      bias_row[m]= sum_o bvec[o] * Bm[m,o]
      S[n,m]     = sum_d A[n,d] * C[m,d] + bias_row[m]   (bias via K=1 matmul)
      P[n,m]     = exp(S - rowmax(S))  (bf16), rowsum via ACT accum_out
      o1[n,d]    = (1/rowsum[n]) * sum_m P^T[m,n] * Bm[m,d]   (P^T via PE transpose)
      o2[m,d]    = sum_n P[n,m] * (A[n,d]/rowsum[n])
    """
    nc = bacc.Bacc("TRN2", target_bir_lowering=False, debug=False, num_devices=B)
    a_d = nc.dram_tensor("a", [N1, D], MM_DT, kind="ExternalInput").ap()
    b_d = nc.dram_tensor("bm", [N2, D], MM_DT, kind="ExternalInput").ap()
    w_d = nc.dram_tensor("w", [D, D], MM_DT, kind="ExternalInput").ap()
    bv_d = nc.dram_tensor("bvec", [D], MM_DT, kind="ExternalInput").ap()
    ones_d = nc.dram_tensor("ones", [1, PT], MM_DT, kind="ExternalInput").ap()
    eye_d = nc.dram_tensor("eye", [PT, PT], MM_DT, kind="ExternalInput").ap()
    o1_d = nc.dram_tensor("o1", [N1, D], F32, kind="ExternalOutput").ap()
    o2_d = nc.dram_tensor("o2", [N2, D], F32, kind="ExternalOutput").ap()

    with tile.TileContext(nc) as tc:
      for rep in range(nreps):
        sfx = f"r{rep}"
        big = ExitStack()
        const = big.enter_context(tc.tile_pool(name=f"const{sfx}", bufs=1))
        ident = const.tile([PT, PT], MM_DT, name="ident", tag="ident")
        nc.sync.dma_start(ident[:], eye_d[:])
        ident_b = const.tile([PT, PT], BF16, name="identb", tag="identb")
        make_identity(nc, ident_b[:])
        ones_row = const.tile([1, PT], MM_DT, name="ones", tag="ones")
        nc.sync.dma_start(ones_row[:], ones_d[:])
        b_col = const.tile([PT, KT], MM_DT, name="bcol", tag="bcol")
        nc.sync.dma_start(b_col[:], bv_d.rearrange("(k p) -> p k", p=PT))
        bias_row = const.tile([1, N2], MM_DT, name="biasrow", tag="biasrow")

        stats = big.enter_context(tc.tile_pool(name=f"stats{sfx}", bufs=1))
        recip_all = stats.tile([PT, NT], F32, name="recip", tag="recip")

        at_pool = big.enter_context(tc.tile_pool(name=f"atp{sfx}", bufs=1))
        AT = [at_pool.tile([PT, N1], MM_DT, name=f"at{k}", tag=f"at{k}") for k in range(KT)]
        ct_pool = big.enter_context(tc.tile_pool(name=f"ctp{sfx}", bufs=1))
        CT = [ct_pool.tile([PT, N2], MM_DT, name=f"ct{k}", tag=f"ct{k}") for k in range(KT)]
        bbf_pool = big.enter_context(tc.tile_pool(name=f"bbfp{sfx}", bufs=1))
        Bbf = [bbf_pool.tile([PT, D], BF16, name=f"bbf{t}", tag=f"bbf{t}") for t in range(MT)]

        # ---------------- phase 0: load + transposes + projection ----------
        with ExitStack() as ph0:
            wp = ph0.enter_context(tc.tile_pool(name=f"wp{sfx}", bufs=1))
            Wt = [wp.tile([PT, D], MM_DT, name=f"w{k}", tag=f"w{k}") for k in range(KT)]
            for k in range(KT):
                nc.sync.dma_start(Wt[k][:], w_d[k * PT:(k + 1) * PT, :])
            btp = ph0.enter_context(tc.tile_pool(name=f"btp{sfx}", bufs=1))
            BT = [btp.tile([PT, N2], MM_DT, name=f"bt{k}", tag=f"bt{k}") for k in range(KT)]
            ldp = ph0.enter_context(tc.tile_pool(name=f"ldp{sfx}", bufs=1))
            ps0 = ph0.enter_context(tc.tile_pool(name=f"ps0{sfx}", bufs=1, space="PSUM"))

            for t in range(MT):
                btile = ldp.tile([PT, D], MM_DT, name="ld", tag="ld", bufs=4)
                nc.sync.dma_start(btile[:], b_d[t * PT:(t + 1) * PT, :])
                nc.vector.tensor_copy(Bbf[t][:], btile[:])
                for k in range(KT):
                    trp = ps0.tile([PT, PT], MM_DT, name="tr", tag="tr", bufs=2)
                    nc.tensor.transpose(trp[:], btile[:, k * PT:(k + 1) * PT], ident[:])
                    nc.vector.tensor_copy(BT[k][:, t * PT:(t + 1) * PT], trp[:])
            for t in range(NT):
                atile = ldp.tile([PT, D], MM_DT, name="ld", tag="ld", bufs=4)
                nc.sync.dma_start(atile[:], a_d[t * PT:(t + 1) * PT, :])
                for k in range(KT):
                    trp = ps0.tile([PT, PT], MM_DT, name="tr", tag="tr", bufs=2)
                    nc.tensor.transpose(trp[:], atile[:, k * PT:(k + 1) * PT], ident[:])
                    nc.vector.tensor_copy(AT[k][:, t * PT:(t + 1) * PT], trp[:])

            # C^T[d2, m] = sum_o W[o, d2] * BT[o, m]
            for k2 in range(KT):
                for mc in range(MC):
                    cps = ps0.tile([PT, CHUNK], F32, name="mm", tag="mm", bufs=5)
                    for ko in range(KT):
                        nc.tensor.matmul(
                            cps[:],
                            Wt[ko][:, k2 * PT:(k2 + 1) * PT],
                            BT[ko][:, mc * CHUNK:(mc + 1) * CHUNK],
                            start=(ko == 0), stop=(ko == KT - 1),
                        )
                    nc.scalar.copy(CT[k2][:, mc * CHUNK:(mc + 1) * CHUNK], cps[:])
            # bias_row[m] = sum_o bvec[o] * BT[o, m]
            for mc in range(MC):
                bps = ps0.tile([1, CHUNK], F32, name="bias", tag="bias", bufs=1)
                for ko in range(KT):
                    nc.tensor.matmul(
                        bps[:],
                        b_col[:, ko:ko + 1],
                        BT[ko][:, mc * CHUNK:(mc + 1) * CHUNK],
                        start=(ko == 0), stop=(ko == KT - 1),
                    )
                nc.vector.tensor_copy(bias_row[0:1, mc * CHUNK:(mc + 1) * CHUNK], bps[:])

        # ---------------- phase 1: S' -> P (bf16), row stats, A_scaled ------
        p_pool = big.enter_context(tc.tile_pool(name=f"pp{sfx}", bufs=1))
        Pt = [p_pool.tile([PT, N2], BF16, name=f"p{t}", tag=f"p{t}") for t in range(NT)]
        asc_pool = big.enter_context(tc.tile_pool(name=f"ascp{sfx}", bufs=1))
        Asc = [asc_pool.tile([PT, D], BF16, name=f"asc{t}", tag=f"asc{t}") for t in range(NT)]
        with ExitStack() as ph1:
            ps1 = ph1.enter_context(tc.tile_pool(name=f"ps1{sfx}", bufs=1, space="PSUM"))
            smp = ph1.enter_context(tc.tile_pool(name=f"smp{sfx}", bufs=1))
            ld1 = ph1.enter_context(tc.tile_pool(name=f"ld1{sfx}", bufs=1))
            for nt in range(NT):
                spsums = []
                for mc in range(MC):
                    sps = ps1.tile([PT, CHUNK], F32, name="s", tag="s", bufs=8)
                    for k in range(KT):
                        nc.tensor.matmul(
                            sps[:],
                            AT[k][:, nt * PT:(nt + 1) * PT],
                            CT[k][:, mc * CHUNK:(mc + 1) * CHUNK],
                            start=(k == 0), stop=False,
                        )
                    nc.tensor.matmul(
                        sps[:],
                        ones_row[:],
                        bias_row[0:1, mc * CHUNK:(mc + 1) * CHUNK],
                        start=False, stop=True,
                    )
                    spsums.append(sps)
                rms = []
                for mc in range(MC):
                    rm = smp.tile([PT, 1], F32, name="rm", tag="rm", bufs=8)
                    nc.vector.tensor_reduce(rm[:], spsums[mc][:], axis=AX.X, op=ALU.max)
                    rms.append(rm)
                c01 = smp.tile([PT, 1], F32, name="c01", tag="c01", bufs=2)
                nc.vector.tensor_tensor(c01[:], rms[0][:], rms[1][:], op=ALU.max)
                c23 = smp.tile([PT, 1], F32, name="c23", tag="c23", bufs=2)
                nc.vector.tensor_tensor(c23[:], rms[2][:], rms[3][:], op=ALU.max)
                mx = smp.tile([PT, 1], F32, name="mx", tag="mx", bufs=2)
                nc.vector.tensor_tensor(mx[:], c01[:], c23[:], op=ALU.max)
                nmax = smp.tile([PT, 1], F32, name="nmax", tag="nmax", bufs=2)
                nc.vector.tensor_scalar_mul(nmax[:], mx[:], -1.0)
                sums = []
                for mc in range(MC):
                    sm = smp.tile([PT, 1], F32, name="sum", tag="sum", bufs=8)
                    nc.scalar.activation(
                        Pt[nt][:, mc * CHUNK:(mc + 1) * CHUNK],
                        spsums[mc][:], AF.Exp,
                        bias=nmax[:], scale=1.0, accum_out=sm[:],
                    )
                    sums.append(sm)
                s01 = smp.tile([PT, 1], F32, name="s01", tag="s01", bufs=2)
                nc.vector.tensor_add(s01[:], sums[0][:], sums[1][:])
                s23 = smp.tile([PT, 1], F32, name="s23", tag="s23", bufs=2)
                nc.vector.tensor_add(s23[:], sums[2][:], sums[3][:])
                stot = smp.tile([PT, 1], F32, name="stot", tag="stot", bufs=2)
                nc.vector.tensor_add(stot[:], s01[:], s23[:])
                nc.vector.reciprocal(recip_all[:, nt:nt + 1], stot[:])
                atile = ld1.tile([PT, D], MM_DT, name="ld", tag="ld", bufs=3)
                nc.sync.dma_start(atile[:], a_d[nt * PT:(nt + 1) * PT, :])
                nc.vector.tensor_scalar_mul(Asc[nt][:], atile[:], recip_all[:, nt:nt + 1])

        # ---------------- pass 2: o1 (needs P^T tiles) and o2 ---------------
        with ExitStack() as ph2:
            ps2 = ph2.enter_context(tc.tile_pool(name=f"ps2{sfx}", bufs=1, space="PSUM"))
            ptp = ph2.enter_context(tc.tile_pool(name=f"ptp{sfx}", bufs=1))
            outp = ph2.enter_context(tc.tile_pool(name=f"outp{sfx}", bufs=1))
            for nt in range(NT):
                o1ps = ps2.tile([PT, D], F32, name="o1", tag="o1", bufs=2)
                for mt in range(MT):
                    trp = ps2.tile([PT, PT], BF16, name="tr", tag="tr", bufs=3)
                    nc.tensor.transpose(trp[:], Pt[nt][:, mt * PT:(mt + 1) * PT], ident_b[:])
                    pts = ptp.tile([PT, PT], BF16, name="pt", tag="pt", bufs=4)
                    nc.vector.tensor_copy(pts[:], trp[:])
                    nc.tensor.matmul(o1ps[:], pts[:], Bbf[mt][:],
                                     start=(mt == 0), stop=(mt == MT - 1))
                o1sb = outp.tile([PT, D], F32, name="o1sb", tag="o1sb", bufs=4)
                nc.scalar.mul(o1sb[:], o1ps[:], recip_all[:, nt:nt + 1])
                nc.sync.dma_start(o1_d[nt * PT:(nt + 1) * PT, :], o1sb[:])
            for mt in range(MT):
                o2ps = ps2.tile([PT, D], F32, name="o2", tag="o2", bufs=2)
                for nt in range(NT):
                    nc.tensor.matmul(o2ps[:], Pt[nt][:, mt * PT:(mt + 1) * PT], Asc[nt][:],
                                     start=(nt == 0), stop=(nt == NT - 1))
                o2sb = outp.tile([PT, D], F32, name="o2sb", tag="o2sb", bufs=4)
                nc.scalar.copy(o2sb[:], o2ps[:])
                nc.sync.dma_start(o2_d[mt * PT:(mt + 1) * PT, :], o2sb[:])
        big.close()
    nc.compile()
    return nc


_state = {}


def _get_nc(nreps=1):
    key = f"nc{nreps}"
    if key not in _state:
        _state[key] = _build(nreps)
    return _state[key]


def _in_maps(input1, input2, W_w, W_b):
    return [
        {
            "a": np.ascontiguousarray(input1[bb], dtype=np.float32),
            "bm": np.ascontiguousarray(input2[bb], dtype=np.float32),
            "w": np.ascontiguousarray(W_w, dtype=np.float32),
            "bvec": np.ascontiguousarray(W_b, dtype=np.float32),
            "ones": np.ones((1, PT), dtype=np.float32),
            "eye": np.eye(PT, dtype=np.float32),
        }
        for bb in range(B)
    ]


def kernel(input1, input2, W_w, W_b):
    res = run_bass_kernel_spmd(
        _get_nc(), _in_maps(input1, input2, W_w, W_b), core_ids=list(range(B))
    )
    o1 = np.stack([r["o1"] for r in res.results])
    o2 = np.stack([r["o2"] for r in res.results])
    return o1, o2


def _pjrt_fn(nc, in_maps, donate=False):
    """Build a single-call jitted runner for `nc` (copy of run_bass_via_pjrt
    multi-core path, without donation so device inputs can be reused)."""
    import jax
    import numpy as np_
    from jax.sharding import Mesh, NamedSharding, PartitionSpec
    from jax.experimental.shard_map import shard_map

    from concourse import mybir as _mybir
    from concourse.bass2jax import (
        _bass_exec_p,
        install_neuronx_cc_hook,
        partition_id_tensor,
    )

    install_neuronx_cc_hook()
    partition_name = nc.partition_id_tensor.name if nc.partition_id_tensor else None

    in_names, out_names, out_avals, zero_outs = [], [], [], []
    for alloc in nc.m.functions[0].allocations:
        if not isinstance(alloc, _mybir.MemoryLocationSet):
            continue
        name = alloc.memorylocations[0].name
        if alloc.kind == "ExternalInput":
            if name != partition_name:
                in_names.append(name)
        elif alloc.kind == "ExternalOutput":
            out_names.append(name)
            shape = tuple(alloc.tensor_shape)
            dtype = _mybir.dt.np(alloc.dtype)
            out_avals.append(jax.core.ShapedArray(shape, dtype))
            zero_outs.append(np_.zeros(shape, dtype))

    all_in = list(in_names) + list(out_names)
    if partition_name is not None:
        all_in.append(partition_name)

    def _body(*args):
        operands = list(args)
        if partition_name is not None:
            operands.append(partition_id_tensor())
        outs = _bass_exec_p.bind(
            *operands,
            out_avals=tuple(out_avals),
            in_names=tuple(all_in),
            out_names=tuple(out_names),
            lowering_input_output_aliases=(),
            sim_require_finite=True,
            sim_require_nnan=True,
            nc=nc,
        )
        return tuple(outs)

    devices = jax.devices()[:B]
    mesh = Mesh(np_.asarray(devices), ("core",))
    nargs = len(in_names) + len(out_names)
    sh = NamedSharding(mesh, PartitionSpec("core"))
    fn = jax.jit(
        shard_map(
            _body, mesh=mesh,
            in_specs=(PartitionSpec("core"),) * nargs,
            out_specs=(PartitionSpec("core"),) * len(out_names),
            check_rep=False,
        ),
        **({"donate_argnums": tuple(range(len(in_names), nargs))} if donate else {}),
    )
    args = [
        jax.device_put(np_.concatenate([m[n] for m in in_maps], axis=0), sh)
        for n in in_names
    ] + [
        jax.device_put(np_.concatenate([z] * B, axis=0), sh) for z in zero_outs
    ]
    return fn, args, out_names, out_avals


def _time_fn(fn, args, calls=30, reps=4):
    """Pipelined timing: issue `calls` executions, block once at the end.
    Returns list of per-call ns (one value per rep)."""
    import time

    import jax

    r = fn(*args)
    jax.block_until_ready(r)
    out = []
    for _ in range(reps):
        t0 = time.perf_counter()
        for _ in range(calls):
            r = fn(*args)
        jax.block_until_ready(r)
        out.append((time.perf_counter() - t0) / calls * 1e9)
    return out


def bench_hw(input1, input2, W_w, W_b, calls=40):
    """HW body time via 2-rep minus 1-rep NEFF wall times (dispatch cancels).
    Returns (body_ns, t1_list_p, t2_list_p)."""
    in_maps = _in_maps(input1, input2, W_w, W_b)
    fn1, args1, _, _ = _pjrt_fn(_get_nc(1), in_maps)
    fn2, args2, _, _ = _pjrt_fn(_get_nc(2), in_maps)
    t1 = _time_fn(fn1, args1, calls)
    t2 = _time_fn(fn2, args2, calls)
    import numpy as np_
    p = lambda ts, q: float(np_.percentile(ts, q))
    body = p(t2, 10) - p(t1, 10)
    return body, (p(t1,10), p(t1,50)), (p(t2,10), p(t2,50))


def bench_floor(iters=30, reps=3):
    """Dispatch floor: identical path with a trivial copy kernel."""
    import time

    import jax

    if "nc_tiny" not in _state:
        nc = bacc.Bacc("TRN2", target_bir_lowering=False, debug=False, num_devices=B)
        x_d = nc.dram_tensor("x", [PT, PT], F32, kind="ExternalInput").ap()
        y_d = nc.dram_tensor("y", [PT, PT], F32, kind="ExternalOutput").ap()
        with tile.TileContext(nc) as tc:
            with tc.tile_pool(name="tp", bufs=1) as tp:
                t = tp.tile([PT, PT], F32, name="t", tag="t")
                nc.sync.dma_start(t[:], x_d[:])
                nc.sync.dma_start(y_d[:], t[:])
        nc.compile()
        _state["nc_tiny"] = nc
    nc = _state["nc_tiny"]
    in_maps = [{"x": np.zeros((PT, PT), np.float32)} for _ in range(B)]
    fn, args, _, _ = _pjrt_fn(nc, in_maps)
    r = fn(*args)
    jax.block_until_ready(r)
    times = []
    for _ in range(reps):
        import time as _t
        t0 = _t.perf_counter()
        for _ in range(iters):
            r = fn(*args)
        jax.block_until_ready(r)
        times.append((_t.perf_counter() - t0) / iters * 1e9)
    return min(times), times
